# revision 1
# baseline (speedup 1.0000x reference)
"""GAT-3-layer distributed Bass kernel for 8 TRN2 NeuronCores.

Algorithm (validated in proto.py, L2 rel err 1.9e-4 vs reference):
- dst-shard nodes: core c owns dst nodes [c*6250, (c+1)*6250)
- per layer: dense phase computes table rows [h (256 cols, c-major head
  interleave: col = 4*c + head) | esrc (4)] bf16 + edst_local [6250, 4];
  AllGather table -> [50000, 260]
- edge phase: per 128-edge tile, indirect-DMA row gather by src, small indirect
  edst gather by dst_local, batched w = exp(leaky_relu(esrc+edst)); cumulative
  selector U[e,d] = (iota_d >= dstcol_e) via one tensor_scalar is_ge; two
  matmuls per tile (w-scaled h + w cols) accumulate per-block CUMULATIVE dst
  sums in PSUM; epilogue recovers per-dst sums by a partition-shift (DMA)
  difference in f32, out = num * recip(z).
- BN stats via ones-lhsT matmuls + AllReduce; affine (with bias folded) applied
  in transposed space fused into the PE-transpose copy; ELU; next projection
  with folded rhs [W | W @ a_flat].
"""
import contextlib
import numpy as np
import ml_dtypes

from concourse import bass, bacc, mybir, tile
from concourse.bass import AP, IndirectOffsetOnAxis

bf16 = ml_dtypes.bfloat16
FP32 = mybir.dt.float32
BF16 = mybir.dt.bfloat16
I32 = mybir.dt.int32
Alu = mybir.AluOpType
Act = mybir.ActivationFunctionType
AxX = mybir.AxisListType.X

N, HEADS = 50000, 4
IN_C, HID_C, OUT_C = 128, 64, 40
NCORES = 8
NLOC = N // NCORES          # 6250
BLK = 128
NBLK = (NLOC + BLK - 1) // BLK   # 49
LAST_ROWS = NLOC - (NBLK - 1) * BLK  # 106
ROWW = 260
G = 64
NEG = 0.2
BN_EPS = 1e-5


def host_prep(x, edge_index, W0, a_src0, a_dst0, b0, g0, beta0,
              W1, a_src1, a_dst1, b1, g1, beta1, W2, a_src2, a_dst2, b2):
    x = np.asarray(x, np.float32)
    ei = np.asarray(edge_index, np.int64)
    src0 = np.concatenate([ei[0], np.arange(N, dtype=np.int64)])
    dst0 = np.concatenate([ei[1], np.arange(N, dtype=np.int64)])

    # in-core degree per local dst, then relabel local ids degree-sorted
    core_of = dst0 // NLOC
    relabel = []      # per core: old_local -> new_local
    inv_relabel = []  # per core: new_local -> old_local
    deg_sorted = []
    for c in range(NCORES):
        m = core_of == c
        dloc_old = (dst0[m] % NLOC).astype(np.int64)
        deg = np.bincount(dloc_old, minlength=NLOC)
        order = np.argsort(deg, kind="stable")     # new id j -> old id order[j]
        inv = np.empty(NLOC, np.int64)
        inv[order] = np.arange(NLOC)
        relabel.append(inv)        # old -> new
        inv_relabel.append(order)  # new -> old
        deg_sorted.append(deg[order])

    # relabeled global src ids
    src = src0.copy()
    for c in range(NCORES):
        m = (src0 >= c * NLOC) & (src0 < (c + 1) * NLOC)
        src[m] = c * NLOC + relabel[c][src0[m] - c * NLOC]

    per_core = []
    for c in range(NCORES):
        m = core_of == c
        s_c = src[m].astype(np.int32)
        d_c = relabel[c][(dst0[m] % NLOC)].astype(np.int32)
        order = np.argsort(d_c, kind="stable")
        s_c, d_c = s_c[order], d_c[order]
        bnds = np.searchsorted(d_c, np.arange(0, NBLK + 1) * BLK)
        per_core.append((s_c, d_c, bnds))

    tiles_per_block = []
    for b in range(NBLK):
        mx = 1
        for c in range(NCORES):
            _, _, bnds = per_core[c]
            n = int(bnds[b + 1] - bnds[b])
            mx = max(mx, (n + BLK - 1) // BLK)
        tiles_per_block.append(mx)
    T = sum(tiles_per_block)
    Tpad = ((T + G - 1) // G) * G
    tiles_per_block[-1] += Tpad - T
    T = Tpad

    metas = []
    rects_all = []   # per core: list of (n0, m, k, e0) expansion rectangles
    for c in range(NCORES):
        s_c, d_c, bnds = per_core[c]
        msrc = np.zeros((T, BLK), np.int32)
        medst = np.zeros((T, BLK), np.int32)
        mdcol = np.full((T, BLK), 200.0, np.float32)
        rects = []
        t0 = 0
        for b in range(NBLK):
            tb = tiles_per_block[b]
            lo, hi = int(bnds[b]), int(bnds[b + 1])
            n = hi - lo
            fs = np.zeros(tb * BLK, np.int32)
            fd = np.zeros(tb * BLK, np.int32)
            fc = np.full(tb * BLK, 200.0, np.float32)
            fs[:n] = s_c[lo:hi]
            fd[:n] = d_c[lo:hi]
            fc[:n] = (d_c[lo:hi] - b * BLK).astype(np.float32)
            msrc[t0:t0 + tb] = fs.reshape(tb, BLK)
            medst[t0:t0 + tb] = fd.reshape(tb, BLK)
            mdcol[t0:t0 + tb] = fc.reshape(tb, BLK)
            # expansion rectangles for this block: nodes n0..n1 (relabeled,
            # degree-sorted so same-degree nodes contiguous), edges contiguous
            # from t0*BLK
            nlo, nhi = b * BLK, min((b + 1) * BLK, NLOC)
            degs = deg_sorted[c][nlo:nhi]
            e_pos = t0 * BLK
            i = 0
            while i < len(degs):
                k = int(degs[i])
                j = i
                while j < len(degs) and degs[j] == k:
                    j += 1
                m_cnt = j - i
                if k > 0:
                    rects.append((nlo + i, m_cnt, k, e_pos))
                    e_pos += m_cnt * k
                i = j
            t0 += tb
        metas.append((np.ascontiguousarray(msrc.T),
                      np.ascontiguousarray(medst.T),
                      np.ascontiguousarray(mdcol.T)))
        rects_all.append(rects)
    nrects = max(len(r) for r in rects_all)
    # pad rect lists to the same count (SPMD): dummy rect (0,1,1,e=pad slot)
    # use a dedicated scratch edge slot: T*BLK (we allocate T*BLK+8 rows)
    for r in rects_all:
        while len(r) < nrects:
            r.append((0, 1, 1, T * BLK))

    def perm_for(out_c):
        p = np.zeros(HEADS * out_c, np.int64)
        for h in range(HEADS):
            for i in range(out_c):
                p[i * HEADS + h] = h * out_c + i
        return p

    p64 = perm_for(HID_C)
    p40 = perm_for(OUT_C)

    def a_flat(a_s, a_d, out_c):
        A = np.zeros((HEADS * out_c, 8), np.float32)
        a_s = np.asarray(a_s, np.float32)
        a_d = np.asarray(a_d, np.float32)
        for h in range(HEADS):
            for i in range(out_c):
                A[i * HEADS + h, h] = a_s[h, i]
                A[i * HEADS + h, 4 + h] = a_d[h, i]
        return A

    W0p = np.asarray(W0, np.float32)[:, p64]
    W0cat = np.concatenate([W0p, W0p @ a_flat(a_src0, a_dst0, HID_C)], 1).astype(bf16)
    W1p = np.asarray(W1, np.float32)[p64][:, p64]
    W1cat = np.concatenate([W1p, W1p @ a_flat(a_src1, a_dst1, HID_C)], 1).astype(bf16)
    W2p = np.asarray(W2, np.float32)[p64][:, p40]
    W2cat = np.concatenate([W2p, W2p @ a_flat(a_src2, a_dst2, OUT_C)], 1).astype(bf16)

    bn0 = np.concatenate([np.asarray(g0, np.float32)[p64],
                          np.asarray(beta0, np.float32)[p64],
                          np.asarray(b0, np.float32)[p64]])[None, :]
    bn1 = np.concatenate([np.asarray(g1, np.float32)[p64],
                          np.asarray(beta1, np.float32)[p64],
                          np.asarray(b1, np.float32)[p64]])[None, :]
    b2row = np.asarray(b2, np.float32)[None, :]

    iota = np.tile(np.arange(128, dtype=np.float32), (128, 1)).astype(bf16)
    ident = np.eye(128, dtype=np.float32).astype(bf16)
    ones_col = np.ones((128, 1), bf16)
    ones11 = np.ones((1, 1), np.float32)

    shared = dict(W0cat=W0cat, W1cat=W1cat, W2cat=W2cat, bn0=bn0, bn1=bn1,
                  b2row=b2row, iota=iota, ident=ident, ones_col=ones_col,
                  ones11=ones11)
    in_maps = []
    for c in range(NCORES):
        msrc, medst, mdcol = metas[c]
        xp = x[c * NLOC:(c + 1) * NLOC][inv_relabel[c]]
        xT = np.ascontiguousarray(xp.T).astype(bf16)
        in_maps.append(dict(xT=xT, msrc=msrc, medst=medst, mdcol=mdcol, **shared))
    return in_maps, T, tiles_per_block, rects_all, inv_relabel


def build(T, tiles_per_block, rects_all, nphases=5, dump=None):
    nc = bacc.Bacc("TRN2", target_bir_lowering=False, debug=False,
                   num_devices=NCORES)

    def din(name, shape, dt):
        return nc.dram_tensor(name, shape, dt, kind="ExternalInput").ap()

    xT = din("xT", [128, NLOC], BF16)
    msrc = din("msrc", [128, T], I32)
    medst = din("medst", [128, T], I32)
    mdcol = din("mdcol", [128, T], FP32)
    W0cat = din("W0cat", [128, 264], BF16)
    W1cat = din("W1cat", [256, 264], BF16)
    W2cat = din("W2cat", [256, 168], BF16)
    bn0 = din("bn0", [1, 768], FP32)
    bn1 = din("bn1", [1, 768], FP32)
    b2row = din("b2row", [1, 40], FP32)
    iota_d = din("iota", [128, 128], BF16)
    ident_d = din("ident", [128, 128], BF16)
    ones_col_d = din("ones_col", [128, 1], BF16)
    ones11_d = din("ones11", [1, 1], FP32)

    out = nc.dram_tensor("out", [NLOC, OUT_C], FP32, kind="ExternalOutput").ap()
    dbg = nc.dram_tensor("dbg", [NLOC, 264], FP32, kind="ExternalOutput").ap() \
        if dump else None

    tile_blk = []
    blk_first, blk_last = {}, {}
    t = 0
    for b, cnt in enumerate(tiles_per_block):
        blk_first[b] = t
        for _ in range(cnt):
            tile_blk.append(b)
            t += 1
        blk_last[b] = t - 1
    assert t == T
    rows_of_blk = [BLK] * (NBLK - 1) + [LAST_ROWS]

    with tile.TileContext(nc) as tc, contextlib.ExitStack() as ctx:
        cpool = ctx.enter_context(tc.tile_pool(name="const", bufs=1))
        dram = ctx.enter_context(tc.tile_pool(name="dram", bufs=1, space="DRAM"))
        gpool = ctx.enter_context(tc.tile_pool(name="gather", bufs=2))
        mpool = ctx.enter_context(tc.tile_pool(name="meta", bufs=2))
        wpool = ctx.enter_context(tc.tile_pool(name="wtile", bufs=2))
        upool = ctx.enter_context(tc.tile_pool(name="usel", bufs=4))
        sclpool = ctx.enter_context(tc.tile_pool(name="scl", bufs=4))
        epool = ctx.enter_context(tc.tile_pool(name="epil", bufs=2))
        opool = ctx.enter_context(tc.tile_pool(name="oblk", bufs=NBLK))
        dpool = ctx.enter_context(tc.tile_pool(name="dense", bufs=3))
        spool = ctx.enter_context(tc.tile_pool(name="small", bufs=4))
        # PSUM pools: worst-case concurrent banks must stay <= 8
        pspool = ctx.enter_context(tc.tile_pool(name="psagg", bufs=2, space="PSUM"))
        stpool = ctx.enter_context(tc.tile_pool(name="psstat", bufs=1, space="PSUM"))
        trpool = ctx.enter_context(tc.tile_pool(name="pstr", bufs=2, space="PSUM"))
        tabps = ctx.enter_context(tc.tile_pool(name="pstab", bufs=1, space="PSUM"))
        afps = ctx.enter_context(tc.tile_pool(name="psaf", bufs=1, space="PSUM"))

        iota_sb = cpool.tile([128, 128], BF16, tag="iota")
        ident_sb = cpool.tile([128, 128], BF16, tag="ident")
        onesc_sb = cpool.tile([128, 1], BF16, tag="onesc")
        ones11_sb = cpool.tile([1, 1], FP32, tag="ones11")
        w0_sb = cpool.tile([128, 264], BF16, tag="w0")
        w1a_sb = cpool.tile([128, 264], BF16, tag="w1a")
        w1b_sb = cpool.tile([128, 264], BF16, tag="w1b")
        w2a_sb = cpool.tile([128, 168], BF16, tag="w2a")
        w2b_sb = cpool.tile([128, 168], BF16, tag="w2b")
        bn0_sb = cpool.tile([1, 768], FP32, tag="bn0")
        bn1_sb = cpool.tile([1, 768], FP32, tag="bn1")
        b2_sb = cpool.tile([1, 40], FP32, tag="b2")
        nc.sync.dma_start(out=iota_sb[:], in_=iota_d[:])
        nc.sync.dma_start(out=ident_sb[:], in_=ident_d[:])
        nc.sync.dma_start(out=onesc_sb[:], in_=ones_col_d[:])
        nc.sync.dma_start(out=ones11_sb[:], in_=ones11_d[:])
        nc.sync.dma_start(out=w0_sb[:], in_=W0cat[:])
        nc.sync.dma_start(out=w1a_sb[:], in_=W1cat[0:128, :])
        nc.sync.dma_start(out=w1b_sb[:], in_=W1cat[128:256, :])
        nc.sync.dma_start(out=w2a_sb[:], in_=W2cat[0:128, :])
        nc.sync.dma_start(out=w2b_sb[:], in_=W2cat[128:256, :])
        nc.sync.dma_start(out=bn0_sb[:], in_=bn0[:])
        nc.sync.dma_start(out=bn1_sb[:], in_=bn1[:])
        nc.sync.dma_start(out=b2_sb[:], in_=b2row[:])

        table_local = dram.tile([NLOC, ROWW], BF16, tag="tloc")
        table_full = dram.tile([N, ROWW], BF16, tag="tfull")
        edst_local = dram.tile([NLOC, 4], BF16, tag="eloc")
        stats_in = dram.tile([1, 512], FP32, tag="sin")
        stats_out = dram.tile([1, 512], FP32, tag="sout")

        rg = [list(range(NCORES))]
        out_blocks = {}
        stats_saved = [None]

        def write_tab(nt, psum_tab, layer):
            rows = rows_of_blk[nt]
            tab = dpool.tile([128, ROWW], BF16, tag="tab")
            eds = spool.tile([128, 4], BF16, tag="eds")
            if layer < 2:
                nc.vector.tensor_copy(out=tab[0:rows, :], in_=psum_tab[0:rows, 0:260])
                nc.vector.tensor_copy(out=eds[0:rows, :], in_=psum_tab[0:rows, 260:264])
            else:
                nc.vector.memset(tab[0:rows, 160:256], 0)
                nc.vector.tensor_copy(out=tab[0:rows, 0:160], in_=psum_tab[0:rows, 0:160])
                nc.vector.tensor_copy(out=tab[0:rows, 256:260],
                                      in_=psum_tab[0:rows, 160:164])
                nc.vector.tensor_copy(out=eds[0:rows, :], in_=psum_tab[0:rows, 164:168])
            nc.sync.dma_start(out=table_local[nt * BLK:nt * BLK + rows, :],
                              in_=tab[0:rows, :])
            nc.sync.dma_start(out=edst_local[nt * BLK:nt * BLK + rows, :],
                              in_=eds[0:rows, :])
            if dump == f"table{layer}":
                tf = dpool.tile([128, 264], FP32, tag="tabf")
                nc.vector.tensor_copy(out=tf[0:rows, 0:260], in_=tab[0:rows, :])
                nc.vector.tensor_copy(out=tf[0:rows, 260:264], in_=eds[0:rows, :])
                nc.sync.dma_start(out=dbg[nt * BLK:nt * BLK + rows, :],
                                  in_=tf[0:rows, :])

        def all_gather_table():
            nc.gpsimd.collective_compute(
                "AllGather", Alu.bypass, replica_groups=rg,
                ins=[table_local[:].opt()], outs=[table_full[:].opt()])

        def epilogue(layer, b, psum, stats_ps, b2e):
            rows = rows_of_blk[b]
            cum = epool.tile([128, ROWW], FP32, tag="cum")
            prev = epool.tile([128, ROWW], FP32, tag="prev")
            diff = epool.tile([128, ROWW], FP32, tag="diff")
            nc.vector.tensor_copy(out=cum[:], in_=psum[:])
            if dump == "edge0" and b == 0 and layer == 0:
                nc.sync.dma_start(out=dbg[640:768, 0:260], in_=cum[:])
            nc.vector.memset(prev[0:1, :], 0)
            nc.sync.dma_start(out=prev[1:128, :], in_=cum[0:127, :])
            nc.vector.tensor_tensor(out=diff[:], in0=cum[:], in1=prev[:],
                                    op=Alu.subtract)
            if dump == "edge0" and b == 0 and layer == 0:
                nc.sync.dma_start(out=dbg[768:896, 0:260], in_=prev[:])
                nc.sync.dma_start(out=dbg[896:1024, 0:260], in_=diff[:])
            if dump == f"agg{layer}":
                nc.sync.dma_start(out=dbg[b * BLK:b * BLK + rows, 0:260],
                                  in_=diff[0:rows, :])
            zr = spool.tile([128, 4], FP32, tag="zr")
            nc.vector.reciprocal(out=zr[0:rows, :], in_=diff[0:rows, 256:260])
            if layer < 2:
                o = opool.tile([128, 256], BF16, tag="ob")
                zrb = AP(zr.tensor, 0, [[4, 128], [0, 64], [1, 4]])
                nc.vector.tensor_tensor(out=o[:], in0=diff[:, 0:256], in1=zrb,
                                        op=Alu.mult)
                sq = spool.tile([128, 256], BF16, tag="sq")
                nc.vector.tensor_tensor(out=sq[0:rows, :], in0=o[0:rows, :],
                                        in1=o[0:rows, :], op=Alu.mult)
                nc.tensor.matmul(stats_ps[0:1, 0:256], lhsT=onesc_sb[0:rows, :],
                                 rhs=o[0:rows, :], start=(b == 0),
                                 stop=(b == NBLK - 1))
                nc.tensor.matmul(stats_ps[0:1, 256:512], lhsT=onesc_sb[0:rows, :],
                                 rhs=sq[0:rows, :], start=(b == 0),
                                 stop=(b == NBLK - 1))
                out_blocks[b] = o
            else:
                nc.vector.tensor_scalar_mul(out=zr[0:rows, :], in0=zr[0:rows, :],
                                            scalar1=0.25)
                m = spool.tile([128, 160], FP32, tag="m2")
                zrb = AP(zr.tensor, 0, [[4, rows], [0, 40], [1, 4]])
                nc.vector.tensor_tensor(out=m[0:rows, :], in0=diff[0:rows, 0:160],
                                        in1=zrb, op=Alu.mult)
                a = spool.tile([128, 80], FP32, tag="a2")
                m0 = AP(m.tensor, 0, [[160, rows], [2, 80]])
                m1 = AP(m.tensor, 1, [[160, rows], [2, 80]])
                nc.vector.tensor_tensor(out=a[0:rows, :], in0=m0, in1=m1, op=Alu.add)
                o40 = spool.tile([128, 40], FP32, tag="o40")
                a0 = AP(a.tensor, 0, [[80, rows], [2, 40]])
                a1 = AP(a.tensor, 1, [[80, rows], [2, 40]])
                nc.vector.tensor_tensor(out=o40[0:rows, :], in0=a0, in1=a1, op=Alu.add)
                nc.vector.tensor_tensor(out=o40[0:rows, :], in0=o40[0:rows, :],
                                        in1=b2e[0:rows, :], op=Alu.add)
                mx = spool.tile([128, 1], FP32, tag="mx")
                nc.vector.tensor_reduce(out=mx[0:rows, :], in_=o40[0:rows, :],
                                        axis=AxX, op=Alu.max)
                nc.vector.tensor_scalar(out=o40[0:rows, :], in0=o40[0:rows, :],
                                        scalar1=mx[0:rows, 0:1], scalar2=None,
                                        op0=Alu.subtract)
                e40 = spool.tile([128, 40], FP32, tag="e40")
                nc.scalar.activation(out=e40[0:rows, :], in_=o40[0:rows, :],
                                     func=Act.Exp)
                sm = spool.tile([128, 1], FP32, tag="sm")
                nc.vector.tensor_reduce(out=sm[0:rows, :], in_=e40[0:rows, :],
                                        axis=AxX, op=Alu.add)
                nc.scalar.activation(out=sm[0:rows, :], in_=sm[0:rows, :], func=Act.Ln)
                nc.vector.tensor_scalar(out=o40[0:rows, :], in0=o40[0:rows, :],
                                        scalar1=sm[0:rows, 0:1], scalar2=None,
                                        op0=Alu.subtract)
                nc.sync.dma_start(out=out[b * BLK:b * BLK + rows, :],
                                  in_=o40[0:rows, :])

        def edge_phase(layer):
            stats_ps = (stpool.tile([1, 512], FP32, space="PSUM", tag="stats",
                                    name="stats") if layer < 2 else None)
            stats_saved[0] = stats_ps
            if layer == 2:
                ps = afps.tile([128, 40], FP32, space="PSUM", tag="psb2")
                b2lh = spool.tile([1, 128], FP32, tag="b2lh")
                nc.vector.memset(b2lh[:], 1.0)
                nc.tensor.matmul(ps[:], lhsT=b2lh[:], rhs=b2_sb[:], start=True,
                                 stop=True)
                b2e = cpool.tile([128, 40], FP32, tag="b2e")
                nc.vector.tensor_copy(out=b2e[:], in_=ps[:])
            else:
                b2e = None
            psum_cur = [None]
            for batch in range(T // G):
                t0 = batch * G
                src_t = mpool.tile([128, G], I32, tag="srci")
                eds_i = mpool.tile([128, G], I32, tag="edsi")
                dcol_t = mpool.tile([128, G], FP32, tag="dcol")
                nc.sync.dma_start(out=src_t[:], in_=msrc[:, t0:t0 + G])
                nc.sync.dma_start(out=eds_i[:], in_=medst[:, t0:t0 + G])
                nc.sync.dma_start(out=dcol_t[:], in_=mdcol[:, t0:t0 + G])
                hg = gpool.tile([128, G * ROWW], BF16, tag="hg")
                edt = wpool.tile([128, G * 4], BF16, tag="edt")
                for g in range(G):
                    hga = AP(hg.tensor, g * ROWW, [[G * ROWW, 128], [1, ROWW]])
                    nc.gpsimd.indirect_dma_start(
                        out=hga, out_offset=None, in_=table_full[:, :],
                        in_offset=IndirectOffsetOnAxis(ap=src_t[:, g:g + 1], axis=0))
                    eda = AP(edt.tensor, g * 4, [[G * 4, 128], [1, 4]])
                    nc.gpsimd.indirect_dma_start(
                        out=eda, out_offset=None, in_=edst_local[:, :],
                        in_offset=IndirectOffsetOnAxis(ap=eds_i[:, g:g + 1], axis=0))
                wt = wpool.tile([128, G * 4], BF16, tag="wt")
                esrc_ap = AP(hg.tensor, 256, [[G * ROWW, 128], [ROWW, G], [1, 4]])
                nc.vector.tensor_tensor(out=wt[:], in0=esrc_ap, in1=edt[:], op=Alu.add)
                wl = wpool.tile([128, G * 4], BF16, tag="wl")
                nc.vector.tensor_scalar_mul(out=wl[:], in0=wt[:], scalar1=NEG)
                nc.vector.tensor_tensor(out=wt[:], in0=wt[:], in1=wl[:], op=Alu.max)
                nc.scalar.activation(out=wt[:], in_=wt[:], func=Act.Exp)
                if dump == "edge0" and batch == 0 and layer == 0:
                    dt_ = dpool.tile([128, 264], FP32, tag="dt_")
                    hg0 = AP(hg.tensor, 0, [[G * ROWW, 128], [1, 260]])
                    nc.vector.tensor_copy(out=dt_[:, 0:260], in_=hg0)
                    nc.sync.dma_start(out=dbg[0:128, :], in_=dt_[:])
                    dt2 = dpool.tile([128, 256], FP32, tag="dt2")
                    nc.vector.tensor_copy(out=dt2[:], in_=wt[:])
                    nc.sync.dma_start(out=dbg[128:256, 0:256], in_=dt2[:])
                    dt3 = dpool.tile([128, 256], FP32, tag="dt3")
                    nc.vector.tensor_copy(out=dt3[:], in_=edt[:])
                    nc.sync.dma_start(out=dbg[512:640, 0:256], in_=dt3[:])
                for g in range(G):
                    t = t0 + g
                    b = tile_blk[t]
                    first, last = blk_first[b] == t, blk_last[b] == t
                    if first:
                        psum_cur[0] = pspool.tile([128, ROWW], FP32, space="PSUM",
                                                  tag="agg", name="agg")
                    psum = psum_cur[0]
                    U = upool.tile([128, 128], BF16, tag="U")
                    dc1 = upool.tile([128, 1], FP32, tag="dc1")
                    nc.vector.tensor_copy(out=dc1[:], in_=dcol_t[:, g:g + 1])
                    nc.vector.tensor_scalar(
                        out=U[:], in0=iota_sb[:], scalar1=dc1[:, 0:1],
                        scalar2=None, op0=Alu.is_ge)
                    scl = sclpool.tile([128, 260], BF16, tag="scl")
                    w_b = AP(wt.tensor, g * 4, [[G * 4, 128], [0, 64], [1, 4]])
                    h_ap = AP(hg.tensor, g * ROWW, [[G * ROWW, 128], [1, 256]])
                    nc.vector.tensor_tensor(out=scl[:, 0:256], in0=h_ap, in1=w_b,
                                            op=Alu.mult)
                    nc.vector.tensor_copy(out=scl[:, 256:260],
                                          in_=wt[:, g * 4:(g + 1) * 4])
                    if dump == "edge0" and t == 1 and layer == 0:
                        dt6 = dpool.tile([128, 128], FP32, tag="dt6")
                        nc.vector.tensor_copy(out=dt6[:], in_=U[:])
                        nc.sync.dma_start(out=dbg[1024:1152, 0:128], in_=dt6[:])
                        dt7 = dpool.tile([128, 260], FP32, tag="dt7")
                        nc.vector.tensor_copy(out=dt7[:], in_=scl[:])
                        nc.sync.dma_start(out=dbg[1152:1280, 0:260], in_=dt7[:])
                    if dump == "edge0" and t == 0 and layer == 0:
                        dt4 = dpool.tile([128, 128], FP32, tag="dt4")
                        nc.vector.tensor_copy(out=dt4[:], in_=U[:])
                        nc.sync.dma_start(out=dbg[256:384, 0:128], in_=dt4[:])
                        dt5 = dpool.tile([128, 260], FP32, tag="dt5")
                        nc.vector.tensor_copy(out=dt5[:], in_=scl[:])
                        nc.sync.dma_start(out=dbg[384:512, 0:260], in_=dt5[:])
                    nc.tensor.matmul(psum[:, 0:260], lhsT=U[:], rhs=scl[:, 0:260],
                                     start=first, stop=last)
                    if last:
                        epilogue(layer, b, psum, stats_ps, b2e)

        def dense_phase(layer):
            bn_sb = bn0_sb if layer == 1 else bn1_sb
            ssb = spool.tile([1, 512], FP32, tag="ssb")
            nc.vector.tensor_copy(out=ssb[:], in_=stats_saved[0][:])
            nc.sync.dma_start(out=stats_in[:], in_=ssb[:])
            nc.gpsimd.collective_compute(
                "AllReduce", Alu.add, replica_groups=rg,
                ins=[stats_in[:].opt()], outs=[stats_out[:].opt()])
            st = spool.tile([1, 512], FP32, tag="st")
            nc.sync.dma_start(out=st[:], in_=stats_out[:])
            mu = spool.tile([1, 256], FP32, tag="mu")
            var = spool.tile([1, 256], FP32, tag="var")
            nc.vector.tensor_scalar_mul(out=mu[:], in0=st[0:1, 0:256], scalar1=1.0 / N)
            nc.vector.tensor_scalar_mul(out=var[:], in0=st[0:1, 256:512],
                                        scalar1=1.0 / N)
            musq = spool.tile([1, 256], FP32, tag="musq")
            nc.vector.tensor_tensor(out=musq[:], in0=mu[:], in1=mu[:], op=Alu.mult)
            nc.vector.tensor_tensor(out=var[:], in0=var[:], in1=musq[:],
                                    op=Alu.subtract)
            sinv = spool.tile([1, 256], FP32, tag="sinv")
            eps_sb = spool.tile([1, 1], FP32, tag="eps")
            nc.vector.memset(eps_sb[:], BN_EPS)
            nc.scalar.activation(out=sinv[:], in_=var[:], func=Act.Ln,
                                 bias=eps_sb[0:1, 0:1])
            nc.scalar.activation(out=sinv[:], in_=sinv[:], func=Act.Exp, scale=-0.5)
            aff = spool.tile([1, 512], FP32, tag="aff")
            nc.vector.tensor_tensor(out=aff[0:1, 0:256], in0=bn_sb[0:1, 0:256],
                                    in1=sinv[:], op=Alu.mult)
            tmp = spool.tile([1, 256], FP32, tag="tmpa")
            nc.vector.tensor_tensor(out=tmp[:], in0=bn_sb[0:1, 512:768], in1=mu[:],
                                    op=Alu.subtract)
            nc.vector.tensor_tensor(out=tmp[:], in0=tmp[:], in1=aff[0:1, 0:256],
                                    op=Alu.mult)
            nc.vector.tensor_tensor(out=aff[0:1, 256:512], in0=bn_sb[0:1, 256:512],
                                    in1=tmp[:], op=Alu.add)
            afc = []
            for ch in range(2):
                ps = afps.tile([128, 2], FP32, space="PSUM", tag="psaf")
                nc.tensor.matmul(ps[:, 0:1], lhsT=aff[0:1, ch * 128:(ch + 1) * 128],
                                 rhs=ones11_sb[:], start=True, stop=True)
                nc.tensor.matmul(ps[:, 1:2],
                                 lhsT=aff[0:1, 256 + ch * 128:256 + (ch + 1) * 128],
                                 rhs=ones11_sb[:], start=True, stop=True)
                sc_c = cpool.tile([128, 1], FP32, tag=f"afsc{layer}_{ch}")
                sh_c = cpool.tile([128, 1], FP32, tag=f"afsh{layer}_{ch}")
                nc.vector.tensor_copy(out=sc_c[:], in_=ps[:, 0:1])
                nc.vector.tensor_copy(out=sh_c[:], in_=ps[:, 1:2])
                afc.append((sc_c, sh_c))

            wA = w1a_sb if layer == 1 else w2a_sb
            wB = w1b_sb if layer == 1 else w2b_sb
            ncols = 264 if layer == 1 else 168
            for nt in range(NBLK):
                o = out_blocks[nt]
                psum_tab = tabps.tile([128, ncols], FP32, space="PSUM", tag="ptab")
                for ch in range(2):
                    psT = trpool.tile([128, 128], BF16, space="PSUM", tag="psT")
                    nc.tensor.transpose(out=psT[:], in_=o[:, ch * 128:(ch + 1) * 128],
                                        identity=ident_sb[:])
                    hT = dpool.tile([128, 128], BF16, tag="hT")
                    nc.vector.tensor_scalar(
                        out=hT[:], in0=psT[:], scalar1=afc[ch][0][:, 0:1],
                        scalar2=afc[ch][1][:, 0:1], op0=Alu.mult, op1=Alu.add)
                    ex = dpool.tile([128, 128], BF16, tag="ex")
                    nc.scalar.activation(out=ex[:], in_=hT[:], func=Act.Exp)
                    nc.vector.tensor_scalar(out=ex[:], in0=ex[:], scalar1=-1.0,
                                            scalar2=0.0, op0=Alu.add, op1=Alu.min)
                    nc.vector.tensor_scalar(out=hT[:], in0=hT[:], scalar1=0.0,
                                            scalar2=None, op0=Alu.max)
                    nc.vector.tensor_tensor(out=hT[:], in0=hT[:], in1=ex[:],
                                            op=Alu.add)
                    nc.tensor.matmul(psum_tab[:], lhsT=hT[:],
                                     rhs=(wA if ch == 0 else wB)[:],
                                     start=(ch == 0), stop=(ch == 1))
                write_tab(nt, psum_tab, layer)

        # ---------------- layer 0 dense ----------------
        for nt in range(NBLK):
            rows = rows_of_blk[nt]
            lx = dpool.tile([128, 128], BF16, tag="lx")
            nc.sync.dma_start(out=lx[:, 0:rows], in_=xT[:, nt * BLK:nt * BLK + rows])
            psum_tab = tabps.tile([128, 264], FP32, space="PSUM", tag="ptab")
            nc.tensor.matmul(psum_tab[0:rows, :], lhsT=lx[:, 0:rows], rhs=w0_sb[:],
                             start=True, stop=True)
            write_tab(nt, psum_tab, 0)

        if nphases >= 2:
            all_gather_table()
        if nphases >= 3:
            edge_phase(0)
        if nphases >= 4:
            dense_phase(1)
            all_gather_table()
            edge_phase(1)
        if nphases >= 5:
            dense_phase(2)
            all_gather_table()
            edge_phase(2)

    nc.compile()
    return nc


# ---------------------------------------------------------------- entry point
_CACHE = {}

def kernel(**inputs):
    """Full (unsharded) inputs -> full [50000, 40] float32 output.

    Shards nodes across 8 NeuronCores by destination (graph parallel),
    compiles and runs the 3-layer GAT Bass kernel SPMD on cores 0-7,
    gathers the per-core output shards.
    """
    from concourse.bass_utils import run_bass_kernel_spmd

    in_maps, T, tpb, rects, inv_rel = host_prep(**inputs)
    key = (T, tuple(tpb))
    if key not in _CACHE:
        _CACHE[key] = build(T, tpb, rects)
    nc = _CACHE[key]
    res = run_bass_kernel_spmd(nc, in_maps, core_ids=list(range(NCORES)),
                               trace=False)
    outs = []
    for c in range(NCORES):
        o = np.asarray(res.results[c]["out"], np.float32)
        un = np.empty_like(o)
        un[inv_rel[c]] = o       # undo the degree relabeling
        outs.append(un)
    return np.concatenate(outs, axis=0).astype(np.float32)


# revision 2
# speedup vs baseline: 1.0100x; 1.0100x over previous
"""GAT-3-layer distributed Bass kernel for 8 TRN2 NeuronCores.

Algorithm (validated in proto.py, L2 rel err 1.9e-4 vs reference):
- dst-shard nodes: core c owns dst nodes [c*6250, (c+1)*6250)
- per layer: dense phase computes table rows [h (256 cols, c-major head
  interleave: col = 4*c + head) | esrc (4)] bf16 + edst_local [6250, 4];
  AllGather table -> [50000, 260]
- edge phase: per 128-edge tile, indirect-DMA row gather by src, small indirect
  edst gather by dst_local, batched w = exp(leaky_relu(esrc+edst)); cumulative
  selector U[e,d] = (iota_d >= dstcol_e) via one tensor_scalar is_ge; two
  matmuls per tile (w-scaled h + w cols) accumulate per-block CUMULATIVE dst
  sums in PSUM; epilogue recovers per-dst sums by a partition-shift (DMA)
  difference in f32, out = num * recip(z).
- BN stats via ones-lhsT matmuls + AllReduce; affine (with bias folded) applied
  in transposed space fused into the PE-transpose copy; ELU; next projection
  with folded rhs [W | W @ a_flat].
"""
import contextlib
import numpy as np
import ml_dtypes

from concourse import bass, bacc, mybir, tile
from concourse.bass import AP, IndirectOffsetOnAxis

bf16 = ml_dtypes.bfloat16
FP32 = mybir.dt.float32
BF16 = mybir.dt.bfloat16
I32 = mybir.dt.int32
Alu = mybir.AluOpType
Act = mybir.ActivationFunctionType
AxX = mybir.AxisListType.X

N, HEADS = 50000, 4
IN_C, HID_C, OUT_C = 128, 64, 40
NCORES = 8
NLOC = N // NCORES          # 6250
BLK = 128
NBLK = (NLOC + BLK - 1) // BLK   # 49
LAST_ROWS = NLOC - (NBLK - 1) * BLK  # 106
ROWW = 260
G = 64
NEG = 0.2
BN_EPS = 1e-5


def host_prep(x, edge_index, W0, a_src0, a_dst0, b0, g0, beta0,
              W1, a_src1, a_dst1, b1, g1, beta1, W2, a_src2, a_dst2, b2):
    x = np.asarray(x, np.float32)
    ei = np.asarray(edge_index, np.int64)
    src0 = np.concatenate([ei[0], np.arange(N, dtype=np.int64)])
    dst0 = np.concatenate([ei[1], np.arange(N, dtype=np.int64)])

    # in-core degree per local dst, then relabel local ids degree-sorted
    core_of = dst0 // NLOC
    relabel = []      # per core: old_local -> new_local
    inv_relabel = []  # per core: new_local -> old_local
    deg_sorted = []
    for c in range(NCORES):
        m = core_of == c
        dloc_old = (dst0[m] % NLOC).astype(np.int64)
        deg = np.bincount(dloc_old, minlength=NLOC)
        order = np.argsort(deg, kind="stable")     # new id j -> old id order[j]
        inv = np.empty(NLOC, np.int64)
        inv[order] = np.arange(NLOC)
        relabel.append(inv)        # old -> new
        inv_relabel.append(order)  # new -> old
        deg_sorted.append(deg[order])

    # relabeled global src ids
    src = src0.copy()
    for c in range(NCORES):
        m = (src0 >= c * NLOC) & (src0 < (c + 1) * NLOC)
        src[m] = c * NLOC + relabel[c][src0[m] - c * NLOC]

    per_core = []
    for c in range(NCORES):
        m = core_of == c
        s_c = src[m].astype(np.int32)
        d_c = relabel[c][(dst0[m] % NLOC)].astype(np.int32)
        order = np.argsort(d_c, kind="stable")
        s_c, d_c = s_c[order], d_c[order]
        bnds = np.searchsorted(d_c, np.arange(0, NBLK + 1) * BLK)
        per_core.append((s_c, d_c, bnds))

    tiles_per_block = []
    for b in range(NBLK):
        mx = 1
        for c in range(NCORES):
            _, _, bnds = per_core[c]
            n = int(bnds[b + 1] - bnds[b])
            mx = max(mx, (n + BLK - 1) // BLK)
        tiles_per_block.append(mx)
    T = sum(tiles_per_block)
    Tpad = ((T + G - 1) // G) * G
    tiles_per_block[-1] += Tpad - T
    T = Tpad

    metas = []
    rects_all = []   # per core: list of (n0, m, k, e0) expansion rectangles
    for c in range(NCORES):
        s_c, d_c, bnds = per_core[c]
        msrc = np.zeros((T, BLK), np.int32)
        medst = np.zeros((T, BLK), np.int32)
        mdcol = np.full((T, BLK), 200.0, np.float32)
        rects = []
        t0 = 0
        for b in range(NBLK):
            tb = tiles_per_block[b]
            lo, hi = int(bnds[b]), int(bnds[b + 1])
            n = hi - lo
            fs = np.zeros(tb * BLK, np.int32)
            fd = np.zeros(tb * BLK, np.int32)
            fc = np.full(tb * BLK, 200.0, np.float32)
            fs[:n] = s_c[lo:hi]
            fd[:n] = d_c[lo:hi]
            fc[:n] = (d_c[lo:hi] - b * BLK).astype(np.float32)
            msrc[t0:t0 + tb] = fs.reshape(tb, BLK)
            medst[t0:t0 + tb] = fd.reshape(tb, BLK)
            mdcol[t0:t0 + tb] = fc.reshape(tb, BLK)
            # expansion rectangles for this block: nodes n0..n1 (relabeled,
            # degree-sorted so same-degree nodes contiguous), edges contiguous
            # from t0*BLK
            nlo, nhi = b * BLK, min((b + 1) * BLK, NLOC)
            degs = deg_sorted[c][nlo:nhi]
            e_pos = t0 * BLK
            i = 0
            while i < len(degs):
                k = int(degs[i])
                j = i
                while j < len(degs) and degs[j] == k:
                    j += 1
                m_cnt = j - i
                if k > 0:
                    rects.append((nlo + i, m_cnt, k, e_pos))
                    e_pos += m_cnt * k
                i = j
            t0 += tb
        metas.append((np.ascontiguousarray(msrc.T),
                      np.ascontiguousarray(medst.T),
                      np.ascontiguousarray(mdcol.T)))
        rects_all.append(rects)
    # scatter-expansion plans: per bit j (1,2,4,...,64): lists of
    # (node_id, write_offset); edge start offsets account for tile padding
    plans_all = []
    for c in range(NCORES):
        s_c, d_c, bnds = per_core[c]
        deg = deg_sorted[c]
        estart = np.zeros(NLOC + 1, np.int64)
        # recompute per-node edge start including padding: block b edges start
        # at (sum tiles before b)*128
        t0 = 0
        for b in range(NBLK):
            nlo, nhi = b * BLK, min((b + 1) * BLK, NLOC)
            base = t0 * BLK
            pref = base + np.concatenate([[0], np.cumsum(deg[nlo:nhi])[:-1]])
            estart[nlo:nhi] = pref
            t0 += tiles_per_block[b]
        plans = {}
        for j in range(7):
            pw = 1 << j
            nodes = np.where((deg & pw) > 0)[0]
            covered = np.zeros(len(nodes), np.int64)
            # offset within node's run: sum of lower set bits
            dd = deg[nodes]
            low = dd & (pw - 1)
            offs = estart[nodes] + low
            plans[pw] = (nodes.astype(np.int32), offs.astype(np.int32))
        plans_all.append(plans)
    # unify counts across cores per pw; pad with scratch writes
    SCRATCH = T * BLK
    plan_arrays = []   # per core: dict pw -> (nodes[128,K], offs[128,K])
    pw_counts = {}
    for pw in [1, 2, 4, 8, 16, 32, 64]:
        mx = max(len(p[pw][0]) for p in plans_all)
        K = (mx + 127) // 128
        if K > 0:
            pw_counts[pw] = K
    for c in range(NCORES):
        d = {}
        for pw in pw_counts:
            K = pw_counts[pw]
            nodes = np.zeros(128 * K, np.int32)
            offs = np.full(128 * K, SCRATCH, np.int32)
            n0, o0 = plans_all[c][pw]
            nodes[:len(n0)] = n0
            offs[:len(o0)] = o0
            # column k = op k: [128] partition-major
            d[pw] = (nodes.reshape(K, 128).T.copy(), offs.reshape(K, 128).T.copy())
        plan_arrays.append(d)

    nrects = max(len(r) for r in rects_all)
    # pad rect lists to the same count (SPMD): dummy rect (0,1,1,e=pad slot)
    # use a dedicated scratch edge slot: T*BLK (we allocate T*BLK+8 rows)
    for r in rects_all:
        while len(r) < nrects:
            r.append((0, 1, 1, T * BLK))

    def perm_for(out_c):
        p = np.zeros(HEADS * out_c, np.int64)
        for h in range(HEADS):
            for i in range(out_c):
                p[i * HEADS + h] = h * out_c + i
        return p

    p64 = perm_for(HID_C)
    p40 = perm_for(OUT_C)

    def a_flat(a_s, a_d, out_c):
        A = np.zeros((HEADS * out_c, 8), np.float32)
        a_s = np.asarray(a_s, np.float32)
        a_d = np.asarray(a_d, np.float32)
        for h in range(HEADS):
            for i in range(out_c):
                A[i * HEADS + h, h] = a_s[h, i]
                A[i * HEADS + h, 4 + h] = a_d[h, i]
        return A

    W0p = np.asarray(W0, np.float32)[:, p64]
    W0cat = np.concatenate([W0p, W0p @ a_flat(a_src0, a_dst0, HID_C)], 1).astype(bf16)
    W1p = np.asarray(W1, np.float32)[p64][:, p64]
    W1cat = np.concatenate([W1p, W1p @ a_flat(a_src1, a_dst1, HID_C)], 1).astype(bf16)
    W2p = np.asarray(W2, np.float32)[p64][:, p40]
    W2cat = np.concatenate([W2p, W2p @ a_flat(a_src2, a_dst2, OUT_C)], 1).astype(bf16)

    bn0 = np.concatenate([np.asarray(g0, np.float32)[p64],
                          np.asarray(beta0, np.float32)[p64],
                          np.asarray(b0, np.float32)[p64]])[None, :]
    bn1 = np.concatenate([np.asarray(g1, np.float32)[p64],
                          np.asarray(beta1, np.float32)[p64],
                          np.asarray(b1, np.float32)[p64]])[None, :]
    b2row = np.asarray(b2, np.float32)[None, :]

    iota = np.tile(np.arange(128, dtype=np.float32), (128, 1)).astype(bf16)
    ident = np.eye(128, dtype=np.float32).astype(bf16)
    ones_col = np.ones((128, 1), bf16)
    ones11 = np.ones((1, 1), np.float32)

    shared = dict(W0cat=W0cat, W1cat=W1cat, W2cat=W2cat, bn0=bn0, bn1=bn1,
                  b2row=b2row, iota=iota, ident=ident, ones_col=ones_col,
                  ones11=ones11)
    in_maps = []
    for c in range(NCORES):
        msrc, medst, mdcol = metas[c]
        xp = x[c * NLOC:(c + 1) * NLOC][inv_relabel[c]]
        xT = np.ascontiguousarray(xp.T).astype(bf16)
        im = dict(xT=xT, msrc=msrc, medst=medst, mdcol=mdcol, **shared)
        for pw in pw_counts:
            nodes, offs = plan_arrays[c][pw]
            im[f"pn{pw}"] = np.ascontiguousarray(nodes)
            im[f"po{pw}"] = np.ascontiguousarray(offs)
        in_maps.append(im)
    return in_maps, T, tiles_per_block, pw_counts, inv_relabel


def build(T, tiles_per_block, pw_counts, nphases=5, dump=None):
    nc = bacc.Bacc("TRN2", target_bir_lowering=False, debug=False,
                   num_devices=NCORES)

    def din(name, shape, dt):
        return nc.dram_tensor(name, shape, dt, kind="ExternalInput").ap()

    xT = din("xT", [128, NLOC], BF16)
    msrc = din("msrc", [128, T], I32)
    medst = din("medst", [128, T], I32)
    mdcol = din("mdcol", [128, T], FP32)
    W0cat = din("W0cat", [128, 264], BF16)
    W1cat = din("W1cat", [256, 264], BF16)
    W2cat = din("W2cat", [256, 168], BF16)
    bn0 = din("bn0", [1, 768], FP32)
    bn1 = din("bn1", [1, 768], FP32)
    b2row = din("b2row", [1, 40], FP32)
    plan_ins = {}
    for pw in pw_counts:
        K = pw_counts[pw]
        plan_ins[pw] = (din(f"pn{pw}", [128, K], I32),
                        din(f"po{pw}", [128, K], I32))
    iota_d = din("iota", [128, 128], BF16)
    ident_d = din("ident", [128, 128], BF16)
    ones_col_d = din("ones_col", [128, 1], BF16)
    ones11_d = din("ones11", [1, 1], FP32)

    out = nc.dram_tensor("out", [NLOC, OUT_C], FP32, kind="ExternalOutput").ap()
    dbg = nc.dram_tensor("dbg", [NLOC, 264], FP32, kind="ExternalOutput").ap() \
        if dump else None

    tile_blk = []
    blk_first, blk_last = {}, {}
    t = 0
    for b, cnt in enumerate(tiles_per_block):
        blk_first[b] = t
        for _ in range(cnt):
            tile_blk.append(b)
            t += 1
        blk_last[b] = t - 1
    assert t == T
    rows_of_blk = [BLK] * (NBLK - 1) + [LAST_ROWS]

    with tile.TileContext(nc) as tc, contextlib.ExitStack() as ctx:
        cpool = ctx.enter_context(tc.tile_pool(name="const", bufs=1))
        dram = ctx.enter_context(tc.tile_pool(name="dram", bufs=1, space="DRAM"))
        gpool = ctx.enter_context(tc.tile_pool(name="gather", bufs=2))
        mpool = ctx.enter_context(tc.tile_pool(name="meta", bufs=2))
        wpool = ctx.enter_context(tc.tile_pool(name="wtile", bufs=2))
        upool = ctx.enter_context(tc.tile_pool(name="usel", bufs=4))
        sclpool = ctx.enter_context(tc.tile_pool(name="scl", bufs=4))
        epool = ctx.enter_context(tc.tile_pool(name="epil", bufs=2))
        opool = ctx.enter_context(tc.tile_pool(name="oblk", bufs=NBLK))
        dpool = ctx.enter_context(tc.tile_pool(name="dense", bufs=3))
        spool = ctx.enter_context(tc.tile_pool(name="small", bufs=4))
        # PSUM pools: worst-case concurrent banks must stay <= 8
        pspool = ctx.enter_context(tc.tile_pool(name="psagg", bufs=2, space="PSUM"))
        stpool = ctx.enter_context(tc.tile_pool(name="psstat", bufs=1, space="PSUM"))
        trpool = ctx.enter_context(tc.tile_pool(name="pstr", bufs=2, space="PSUM"))
        tabps = ctx.enter_context(tc.tile_pool(name="pstab", bufs=1, space="PSUM"))
        afps = ctx.enter_context(tc.tile_pool(name="psaf", bufs=1, space="PSUM"))

        iota_sb = cpool.tile([128, 128], BF16, tag="iota")
        ident_sb = cpool.tile([128, 128], BF16, tag="ident")
        onesc_sb = cpool.tile([128, 1], BF16, tag="onesc")
        ones11_sb = cpool.tile([1, 1], FP32, tag="ones11")
        w0_sb = cpool.tile([128, 264], BF16, tag="w0")
        w1a_sb = cpool.tile([128, 264], BF16, tag="w1a")
        w1b_sb = cpool.tile([128, 264], BF16, tag="w1b")
        w2a_sb = cpool.tile([128, 168], BF16, tag="w2a")
        w2b_sb = cpool.tile([128, 168], BF16, tag="w2b")
        bn0_sb = cpool.tile([1, 768], FP32, tag="bn0")
        bn1_sb = cpool.tile([1, 768], FP32, tag="bn1")
        b2_sb = cpool.tile([1, 40], FP32, tag="b2")
        nc.sync.dma_start(out=iota_sb[:], in_=iota_d[:])
        nc.sync.dma_start(out=ident_sb[:], in_=ident_d[:])
        nc.sync.dma_start(out=onesc_sb[:], in_=ones_col_d[:])
        nc.sync.dma_start(out=ones11_sb[:], in_=ones11_d[:])
        nc.sync.dma_start(out=w0_sb[:], in_=W0cat[:])
        nc.sync.dma_start(out=w1a_sb[:], in_=W1cat[0:128, :])
        nc.sync.dma_start(out=w1b_sb[:], in_=W1cat[128:256, :])
        nc.sync.dma_start(out=w2a_sb[:], in_=W2cat[0:128, :])
        nc.sync.dma_start(out=w2b_sb[:], in_=W2cat[128:256, :])
        nc.sync.dma_start(out=bn0_sb[:], in_=bn0[:])
        nc.sync.dma_start(out=bn1_sb[:], in_=bn1[:])
        nc.sync.dma_start(out=b2_sb[:], in_=b2row[:])

        table_local = dram.tile([NLOC, ROWW], BF16, tag="tloc")
        table_full = dram.tile([N, ROWW], BF16, tag="tfull")
        edst_local = dram.tile([NLOC, 4], BF16, tag="eloc")
        edst_pe = dram.tile([T * BLK + 128, 4], BF16, tag="epe")
        zcols = (T * BLK + 128) * 4 // 128
        zed = cpool.tile([128, zcols], BF16, tag="zed")
        nc.vector.memset(zed[:], 0)
        nc.sync.dma_start(out=edst_pe[:], in_=zed[:])
        stats_in = dram.tile([1, 512], FP32, tag="sin")
        stats_out = dram.tile([1, 512], FP32, tag="sout")

        rg = [list(range(NCORES))]
        out_blocks = {}
        stats_saved = [None]

        def write_tab(nt, psum_tab, layer):
            rows = rows_of_blk[nt]
            tab = dpool.tile([128, ROWW], BF16, tag="tab")
            eds = spool.tile([128, 4], BF16, tag="eds")
            if layer < 2:
                nc.vector.tensor_copy(out=tab[0:rows, :], in_=psum_tab[0:rows, 0:260])
                nc.vector.tensor_copy(out=eds[0:rows, :], in_=psum_tab[0:rows, 260:264])
            else:
                nc.vector.memset(tab[0:rows, 160:256], 0)
                nc.vector.tensor_copy(out=tab[0:rows, 0:160], in_=psum_tab[0:rows, 0:160])
                nc.vector.tensor_copy(out=tab[0:rows, 256:260],
                                      in_=psum_tab[0:rows, 160:164])
                nc.vector.tensor_copy(out=eds[0:rows, :], in_=psum_tab[0:rows, 164:168])
            nc.sync.dma_start(out=table_local[nt * BLK:nt * BLK + rows, :],
                              in_=tab[0:rows, :])
            nc.sync.dma_start(out=edst_local[nt * BLK:nt * BLK + rows, :],
                              in_=eds[0:rows, :])
            if dump == f"table{layer}":
                tf = dpool.tile([128, 264], FP32, tag="tabf")
                nc.vector.tensor_copy(out=tf[0:rows, 0:260], in_=tab[0:rows, :])
                nc.vector.tensor_copy(out=tf[0:rows, 260:264], in_=eds[0:rows, :])
                nc.sync.dma_start(out=dbg[nt * BLK:nt * BLK + rows, :],
                                  in_=tf[0:rows, :])

        def expand_edst():
            for pw in pw_counts:
                K = pw_counts[pw]
                pn_d, po_d = plan_ins[pw]
                pn = mpool.tile([128, K], I32, tag=f"pn{pw}", name=f"pn{pw}")
                po = mpool.tile([128, K], I32, tag=f"po{pw}", name=f"po{pw}")
                nc.sync.dma_start(out=pn[:], in_=pn_d[:])
                nc.sync.dma_start(out=po[:], in_=po_d[:])
                for k in range(K):
                    nrow = sclpool.tile([128, 4], BF16, tag="nrow", name="nrow")
                    nc.gpsimd.indirect_dma_start(
                        out=nrow[:], out_offset=None, in_=edst_local[:, :],
                        in_offset=IndirectOffsetOnAxis(ap=pn[:, k:k + 1], axis=0))
                    rep = sclpool.tile([128, pw * 4], BF16, tag=f"rep{pw}",
                                       name=f"rep{pw}")
                    rbc = AP(nrow.tensor, 0, [[4, 128], [0, pw], [1, 4]])
                    nc.vector.tensor_copy(out=rep[:], in_=rbc)
                    nc.gpsimd.indirect_dma_start(
                        out=edst_pe[:], out_offset=IndirectOffsetOnAxis(
                            ap=po[:, k:k + 1], axis=0),
                        in_=rep[:], in_offset=None)

        def all_gather_table():
            nc.gpsimd.collective_compute(
                "AllGather", Alu.bypass, replica_groups=rg,
                ins=[table_local[:].opt()], outs=[table_full[:].opt()])

        def epilogue(layer, b, psum, stats_ps, b2e):
            rows = rows_of_blk[b]
            cum = epool.tile([128, ROWW], FP32, tag="cum")
            prev = epool.tile([128, ROWW], FP32, tag="prev")
            diff = epool.tile([128, ROWW], FP32, tag="diff")
            nc.vector.tensor_copy(out=cum[:], in_=psum[:])
            if dump == "edge0" and b == 0 and layer == 0:
                nc.sync.dma_start(out=dbg[640:768, 0:260], in_=cum[:])
            nc.vector.memset(prev[0:1, :], 0)
            nc.sync.dma_start(out=prev[1:128, :], in_=cum[0:127, :])
            nc.vector.tensor_tensor(out=diff[:], in0=cum[:], in1=prev[:],
                                    op=Alu.subtract)
            if dump == "edge0" and b == 0 and layer == 0:
                nc.sync.dma_start(out=dbg[768:896, 0:260], in_=prev[:])
                nc.sync.dma_start(out=dbg[896:1024, 0:260], in_=diff[:])
            if dump == f"agg{layer}":
                nc.sync.dma_start(out=dbg[b * BLK:b * BLK + rows, 0:260],
                                  in_=diff[0:rows, :])
            zr = spool.tile([128, 4], FP32, tag="zr")
            nc.vector.reciprocal(out=zr[0:rows, :], in_=diff[0:rows, 256:260])
            if layer < 2:
                o = opool.tile([128, 256], BF16, tag="ob")
                zrb = AP(zr.tensor, 0, [[4, 128], [0, 64], [1, 4]])
                nc.vector.tensor_tensor(out=o[:], in0=diff[:, 0:256], in1=zrb,
                                        op=Alu.mult)
                sq = spool.tile([128, 256], BF16, tag="sq")
                nc.vector.tensor_tensor(out=sq[0:rows, :], in0=o[0:rows, :],
                                        in1=o[0:rows, :], op=Alu.mult)
                nc.tensor.matmul(stats_ps[0:1, 0:256], lhsT=onesc_sb[0:rows, :],
                                 rhs=o[0:rows, :], start=(b == 0),
                                 stop=(b == NBLK - 1))
                nc.tensor.matmul(stats_ps[0:1, 256:512], lhsT=onesc_sb[0:rows, :],
                                 rhs=sq[0:rows, :], start=(b == 0),
                                 stop=(b == NBLK - 1))
                out_blocks[b] = o
            else:
                nc.vector.tensor_scalar_mul(out=zr[0:rows, :], in0=zr[0:rows, :],
                                            scalar1=0.25)
                m = spool.tile([128, 160], FP32, tag="m2")
                zrb = AP(zr.tensor, 0, [[4, rows], [0, 40], [1, 4]])
                nc.vector.tensor_tensor(out=m[0:rows, :], in0=diff[0:rows, 0:160],
                                        in1=zrb, op=Alu.mult)
                a = spool.tile([128, 80], FP32, tag="a2")
                m0 = AP(m.tensor, 0, [[160, rows], [2, 80]])
                m1 = AP(m.tensor, 1, [[160, rows], [2, 80]])
                nc.vector.tensor_tensor(out=a[0:rows, :], in0=m0, in1=m1, op=Alu.add)
                o40 = spool.tile([128, 40], FP32, tag="o40")
                a0 = AP(a.tensor, 0, [[80, rows], [2, 40]])
                a1 = AP(a.tensor, 1, [[80, rows], [2, 40]])
                nc.vector.tensor_tensor(out=o40[0:rows, :], in0=a0, in1=a1, op=Alu.add)
                nc.vector.tensor_tensor(out=o40[0:rows, :], in0=o40[0:rows, :],
                                        in1=b2e[0:rows, :], op=Alu.add)
                mx = spool.tile([128, 1], FP32, tag="mx")
                nc.vector.tensor_reduce(out=mx[0:rows, :], in_=o40[0:rows, :],
                                        axis=AxX, op=Alu.max)
                nc.vector.tensor_scalar(out=o40[0:rows, :], in0=o40[0:rows, :],
                                        scalar1=mx[0:rows, 0:1], scalar2=None,
                                        op0=Alu.subtract)
                e40 = spool.tile([128, 40], FP32, tag="e40")
                nc.scalar.activation(out=e40[0:rows, :], in_=o40[0:rows, :],
                                     func=Act.Exp)
                sm = spool.tile([128, 1], FP32, tag="sm")
                nc.vector.tensor_reduce(out=sm[0:rows, :], in_=e40[0:rows, :],
                                        axis=AxX, op=Alu.add)
                nc.scalar.activation(out=sm[0:rows, :], in_=sm[0:rows, :], func=Act.Ln)
                nc.vector.tensor_scalar(out=o40[0:rows, :], in0=o40[0:rows, :],
                                        scalar1=sm[0:rows, 0:1], scalar2=None,
                                        op0=Alu.subtract)
                nc.sync.dma_start(out=out[b * BLK:b * BLK + rows, :],
                                  in_=o40[0:rows, :])

        def edge_phase(layer):
            stats_ps = (stpool.tile([1, 512], FP32, space="PSUM", tag="stats",
                                    name="stats") if layer < 2 else None)
            stats_saved[0] = stats_ps
            if layer == 2:
                ps = afps.tile([128, 40], FP32, space="PSUM", tag="psb2")
                b2lh = spool.tile([1, 128], FP32, tag="b2lh")
                nc.vector.memset(b2lh[:], 1.0)
                nc.tensor.matmul(ps[:], lhsT=b2lh[:], rhs=b2_sb[:], start=True,
                                 stop=True)
                b2e = cpool.tile([128, 40], FP32, tag="b2e")
                nc.vector.tensor_copy(out=b2e[:], in_=ps[:])
            else:
                b2e = None
            psum_cur = [None]
            for batch in range(T // G):
                t0 = batch * G
                src_t = mpool.tile([128, G], I32, tag="srci")
                eds_i = mpool.tile([128, G], I32, tag="edsi")
                dcol_t = mpool.tile([128, G], FP32, tag="dcol")
                nc.sync.dma_start(out=src_t[:], in_=msrc[:, t0:t0 + G])
                nc.sync.dma_start(out=eds_i[:], in_=medst[:, t0:t0 + G])
                nc.sync.dma_start(out=dcol_t[:], in_=mdcol[:, t0:t0 + G])
                hg = gpool.tile([128, G * ROWW], BF16, tag="hg")
                edt = wpool.tile([128, G * 4], BF16, tag="edt")
                for g in range(G):
                    hga = AP(hg.tensor, g * ROWW, [[G * ROWW, 128], [1, ROWW]])
                    nc.gpsimd.indirect_dma_start(
                        out=hga, out_offset=None, in_=table_full[:, :],
                        in_offset=IndirectOffsetOnAxis(ap=src_t[:, g:g + 1], axis=0))
                epa = AP(edst_pe.tensor, t0 * BLK * 4,
                         [[4, 128], [BLK * 4, G], [1, 4]])
                nc.sync.dma_start(out=edt[:], in_=epa)
                wt = wpool.tile([128, G * 4], BF16, tag="wt")
                esrc_ap = AP(hg.tensor, 256, [[G * ROWW, 128], [ROWW, G], [1, 4]])
                nc.vector.tensor_tensor(out=wt[:], in0=esrc_ap, in1=edt[:], op=Alu.add)
                wl = wpool.tile([128, G * 4], BF16, tag="wl")
                nc.vector.tensor_scalar_mul(out=wl[:], in0=wt[:], scalar1=NEG)
                nc.vector.tensor_tensor(out=wt[:], in0=wt[:], in1=wl[:], op=Alu.max)
                nc.scalar.activation(out=wt[:], in_=wt[:], func=Act.Exp)
                if dump == "edge0" and batch == 0 and layer == 0:
                    dt_ = dpool.tile([128, 264], FP32, tag="dt_")
                    hg0 = AP(hg.tensor, 0, [[G * ROWW, 128], [1, 260]])
                    nc.vector.tensor_copy(out=dt_[:, 0:260], in_=hg0)
                    nc.sync.dma_start(out=dbg[0:128, :], in_=dt_[:])
                    dt2 = dpool.tile([128, 256], FP32, tag="dt2")
                    nc.vector.tensor_copy(out=dt2[:], in_=wt[:])
                    nc.sync.dma_start(out=dbg[128:256, 0:256], in_=dt2[:])
                    dt3 = dpool.tile([128, 256], FP32, tag="dt3")
                    nc.vector.tensor_copy(out=dt3[:], in_=edt[:])
                    nc.sync.dma_start(out=dbg[512:640, 0:256], in_=dt3[:])
                for g in range(G):
                    t = t0 + g
                    b = tile_blk[t]
                    first, last = blk_first[b] == t, blk_last[b] == t
                    if first:
                        psum_cur[0] = pspool.tile([128, ROWW], FP32, space="PSUM",
                                                  tag="agg", name="agg")
                    psum = psum_cur[0]
                    U = upool.tile([128, 128], BF16, tag="U")
                    dc1 = upool.tile([128, 1], FP32, tag="dc1")
                    nc.vector.tensor_copy(out=dc1[:], in_=dcol_t[:, g:g + 1])
                    nc.vector.tensor_scalar(
                        out=U[:], in0=iota_sb[:], scalar1=dc1[:, 0:1],
                        scalar2=None, op0=Alu.is_ge)
                    scl = sclpool.tile([128, 260], BF16, tag="scl")
                    w_b = AP(wt.tensor, g * 4, [[G * 4, 128], [0, 64], [1, 4]])
                    h_ap = AP(hg.tensor, g * ROWW, [[G * ROWW, 128], [1, 256]])
                    nc.vector.tensor_tensor(out=scl[:, 0:256], in0=h_ap, in1=w_b,
                                            op=Alu.mult)
                    nc.vector.tensor_copy(out=scl[:, 256:260],
                                          in_=wt[:, g * 4:(g + 1) * 4])
                    if dump == "edge0" and t == 1 and layer == 0:
                        dt6 = dpool.tile([128, 128], FP32, tag="dt6")
                        nc.vector.tensor_copy(out=dt6[:], in_=U[:])
                        nc.sync.dma_start(out=dbg[1024:1152, 0:128], in_=dt6[:])
                        dt7 = dpool.tile([128, 260], FP32, tag="dt7")
                        nc.vector.tensor_copy(out=dt7[:], in_=scl[:])
                        nc.sync.dma_start(out=dbg[1152:1280, 0:260], in_=dt7[:])
                    if dump == "edge0" and t == 0 and layer == 0:
                        dt4 = dpool.tile([128, 128], FP32, tag="dt4")
                        nc.vector.tensor_copy(out=dt4[:], in_=U[:])
                        nc.sync.dma_start(out=dbg[256:384, 0:128], in_=dt4[:])
                        dt5 = dpool.tile([128, 260], FP32, tag="dt5")
                        nc.vector.tensor_copy(out=dt5[:], in_=scl[:])
                        nc.sync.dma_start(out=dbg[384:512, 0:260], in_=dt5[:])
                    nc.tensor.matmul(psum[:, 0:260], lhsT=U[:], rhs=scl[:, 0:260],
                                     start=first, stop=last)
                    if last:
                        epilogue(layer, b, psum, stats_ps, b2e)

        def dense_phase(layer):
            bn_sb = bn0_sb if layer == 1 else bn1_sb
            ssb = spool.tile([1, 512], FP32, tag="ssb")
            nc.vector.tensor_copy(out=ssb[:], in_=stats_saved[0][:])
            nc.sync.dma_start(out=stats_in[:], in_=ssb[:])
            nc.gpsimd.collective_compute(
                "AllReduce", Alu.add, replica_groups=rg,
                ins=[stats_in[:].opt()], outs=[stats_out[:].opt()])
            st = spool.tile([1, 512], FP32, tag="st")
            nc.sync.dma_start(out=st[:], in_=stats_out[:])
            mu = spool.tile([1, 256], FP32, tag="mu")
            var = spool.tile([1, 256], FP32, tag="var")
            nc.vector.tensor_scalar_mul(out=mu[:], in0=st[0:1, 0:256], scalar1=1.0 / N)
            nc.vector.tensor_scalar_mul(out=var[:], in0=st[0:1, 256:512],
                                        scalar1=1.0 / N)
            musq = spool.tile([1, 256], FP32, tag="musq")
            nc.vector.tensor_tensor(out=musq[:], in0=mu[:], in1=mu[:], op=Alu.mult)
            nc.vector.tensor_tensor(out=var[:], in0=var[:], in1=musq[:],
                                    op=Alu.subtract)
            sinv = spool.tile([1, 256], FP32, tag="sinv")
            eps_sb = spool.tile([1, 1], FP32, tag="eps")
            nc.vector.memset(eps_sb[:], BN_EPS)
            nc.scalar.activation(out=sinv[:], in_=var[:], func=Act.Ln,
                                 bias=eps_sb[0:1, 0:1])
            nc.scalar.activation(out=sinv[:], in_=sinv[:], func=Act.Exp, scale=-0.5)
            aff = spool.tile([1, 512], FP32, tag="aff")
            nc.vector.tensor_tensor(out=aff[0:1, 0:256], in0=bn_sb[0:1, 0:256],
                                    in1=sinv[:], op=Alu.mult)
            tmp = spool.tile([1, 256], FP32, tag="tmpa")
            nc.vector.tensor_tensor(out=tmp[:], in0=bn_sb[0:1, 512:768], in1=mu[:],
                                    op=Alu.subtract)
            nc.vector.tensor_tensor(out=tmp[:], in0=tmp[:], in1=aff[0:1, 0:256],
                                    op=Alu.mult)
            nc.vector.tensor_tensor(out=aff[0:1, 256:512], in0=bn_sb[0:1, 256:512],
                                    in1=tmp[:], op=Alu.add)
            afc = []
            for ch in range(2):
                ps = afps.tile([128, 2], FP32, space="PSUM", tag="psaf")
                nc.tensor.matmul(ps[:, 0:1], lhsT=aff[0:1, ch * 128:(ch + 1) * 128],
                                 rhs=ones11_sb[:], start=True, stop=True)
                nc.tensor.matmul(ps[:, 1:2],
                                 lhsT=aff[0:1, 256 + ch * 128:256 + (ch + 1) * 128],
                                 rhs=ones11_sb[:], start=True, stop=True)
                sc_c = cpool.tile([128, 1], FP32, tag=f"afsc{layer}_{ch}")
                sh_c = cpool.tile([128, 1], FP32, tag=f"afsh{layer}_{ch}")
                nc.vector.tensor_copy(out=sc_c[:], in_=ps[:, 0:1])
                nc.vector.tensor_copy(out=sh_c[:], in_=ps[:, 1:2])
                afc.append((sc_c, sh_c))

            wA = w1a_sb if layer == 1 else w2a_sb
            wB = w1b_sb if layer == 1 else w2b_sb
            ncols = 264 if layer == 1 else 168
            for nt in range(NBLK):
                o = out_blocks[nt]
                psum_tab = tabps.tile([128, ncols], FP32, space="PSUM", tag="ptab")
                for ch in range(2):
                    psT = trpool.tile([128, 128], BF16, space="PSUM", tag="psT")
                    nc.tensor.transpose(out=psT[:], in_=o[:, ch * 128:(ch + 1) * 128],
                                        identity=ident_sb[:])
                    hT = dpool.tile([128, 128], BF16, tag="hT")
                    nc.vector.tensor_scalar(
                        out=hT[:], in0=psT[:], scalar1=afc[ch][0][:, 0:1],
                        scalar2=afc[ch][1][:, 0:1], op0=Alu.mult, op1=Alu.add)
                    ex = dpool.tile([128, 128], BF16, tag="ex")
                    nc.scalar.activation(out=ex[:], in_=hT[:], func=Act.Exp)
                    nc.vector.tensor_scalar(out=ex[:], in0=ex[:], scalar1=-1.0,
                                            scalar2=0.0, op0=Alu.add, op1=Alu.min)
                    nc.vector.tensor_scalar(out=hT[:], in0=hT[:], scalar1=0.0,
                                            scalar2=None, op0=Alu.max)
                    nc.vector.tensor_tensor(out=hT[:], in0=hT[:], in1=ex[:],
                                            op=Alu.add)
                    nc.tensor.matmul(psum_tab[:], lhsT=hT[:],
                                     rhs=(wA if ch == 0 else wB)[:],
                                     start=(ch == 0), stop=(ch == 1))
                write_tab(nt, psum_tab, layer)

        # ---------------- layer 0 dense ----------------
        for nt in range(NBLK):
            rows = rows_of_blk[nt]
            lx = dpool.tile([128, 128], BF16, tag="lx")
            nc.sync.dma_start(out=lx[:, 0:rows], in_=xT[:, nt * BLK:nt * BLK + rows])
            psum_tab = tabps.tile([128, 264], FP32, space="PSUM", tag="ptab")
            nc.tensor.matmul(psum_tab[0:rows, :], lhsT=lx[:, 0:rows], rhs=w0_sb[:],
                             start=True, stop=True)
            write_tab(nt, psum_tab, 0)

        if nphases >= 2:
            expand_edst()
            all_gather_table()
        if nphases >= 3:
            edge_phase(0)
        if nphases >= 4:
            dense_phase(1)
            expand_edst()
            all_gather_table()
            edge_phase(1)
        if nphases >= 5:
            dense_phase(2)
            expand_edst()
            all_gather_table()
            edge_phase(2)

    nc.compile()
    return nc


# ---------------------------------------------------------------- entry point
_CACHE = {}

def kernel(**inputs):
    """Full (unsharded) inputs -> full [50000, 40] float32 output.

    Dst-shards nodes across the 8 NeuronCores (graph parallel), compiles and
    runs the 3-layer GAT Bass kernel SPMD on cores 0-7 with AllGather halo
    exchange of the projected node tables, gathers the per-core output shards.
    """
    from concourse.bass_utils import run_bass_kernel_spmd

    in_maps, T, tpb, pw_counts, inv_rel = host_prep(**inputs)
    key = (T, tuple(tpb), tuple(sorted(pw_counts.items())))
    if key not in _CACHE:
        _CACHE[key] = build(T, tpb, pw_counts)
    nc = _CACHE[key]
    res = run_bass_kernel_spmd(nc, in_maps, core_ids=list(range(NCORES)),
                               trace=False)
    outs = []
    for c in range(NCORES):
        o = np.asarray(res.results[c]["out"], np.float32)
        un = np.empty_like(o)
        un[inv_rel[c]] = o       # undo the degree relabeling
        outs.append(un)
    return np.concatenate(outs, axis=0).astype(np.float32)
